# revision 43
# baseline (speedup 1.0000x reference)
"""Trainium2 Bass kernel for nn_NodeRNN (masked single-step LSTM over N nodes).

Strategy: the reference only *computes* on active rows (ts_mask==1, ~50%) and
passes old state through elsewhere. The host gathers the active rows, packs
them feature-major into per-core DRAM images (xt in fp8e4m3, state in bf16),
the device runs a dense unmasked LSTM step on the gathered rows, and the host
scatters results back (inactive rows are exact f32 passthrough).

Device (per core, CAP_PC=16896 rows = 16 x 1024 + 1 x 512 runt block):
  x.T = relu(W_hid @ feats + W_pos @ xv + biases):
    aux matmul [3,128] (rows x0,x1,1.0) folds W_pos AND both biases, then the
    512-feature contraction runs as 2 fp8 DoubleRow matmuls (2 chunks/pass),
    relu is a DVE tensor_scalar_max out of PSUM.                      (PE+DVE)
  gates j: W_hh.T_j @ hv + W_ih.T_j @ x in bf16 -> sigmoid/tanh + bias (PE+ACT)
  c = f*cv + i*g; h = o*tanh(c) as bf16 tensor_tensor ops; tanh + h run in
    2-block slabs deferred one block (deps always ready); out-DMA per
    4-block superblock from the GpSimd queue.                        (DVE+ACT)
"""
import sys

sys.path.insert(0, "/opt/trn_rl_repo")

import ml_dtypes
import numpy as np

import concourse.bacc as bacc
import concourse.tile as tile
from concourse import mybir
from concourse.bass_utils import run_bass_kernel_spmd

f32 = mybir.dt.float32
bf16 = mybir.dt.bfloat16
f8 = mybir.dt.float8e4
AF = mybir.ActivationFunctionType
DR = mybir.MatmulPerfMode.DoubleRow
nbf16 = ml_dtypes.bfloat16
nf8 = ml_dtypes.float8_e4m3fn

N = 262144
NCORES = 8
BLOCKS = [512] + [1024] * 16  # small first block -> short pipeline fill
NBLK = len(BLOCKS)
NOFF = np.cumsum([0] + BLOCKS)
CAP_PC = int(NOFF[-1])        # 16896 gathered rows per core
CAP = CAP_PC * NCORES         # 135168 total (active ~131072, +16 sigma)
SUPERS = [1, 4, 4, 4, 4]      # blocks per superblock (output granularity)
SUP_BLK0 = np.cumsum([0] + SUPERS)
EMBED = 64
EDGE_H = 256
NODE_H = 128

# cst (bf16) free-dim offsets
CO_WIH = 0                    # W_ih.T [128, 512]
CO_WHH = 512                  # W_hh.T [128, 512]
CO_BG = 1024                  # (b_ih + b_hh) as [128, 4], col j = gate chunk j
CO_WP = 1028                  # [3, 128]: rows 0:2 = [W_pos.T | 0], row 2 = [b_pos | b_hid]
CF = 1156

GATE_FUNCS = [AF.Sigmoid, AF.Sigmoid, AF.Tanh, AF.Sigmoid]  # i, f, g, o

_cached = {}


def build_nc():
    nc = bacc.Bacc(target_bir_lowering=False)
    blk_d = nc.dram_tensor("blk", [128, 8 * CAP_PC], mybir.dt.uint8,
                           kind="ExternalInput")
    aux_d = nc.dram_tensor("aux", [3, CAP_PC], bf16, kind="ExternalInput")
    cst_d = nc.dram_tensor("cst", [128, CF], bf16, kind="ExternalInput")
    cst8_d = nc.dram_tensor("cst8", [128, 4 * EMBED], f8, kind="ExternalInput")
    out_d = nc.dram_tensor("hc_out", [128, 2 * CAP_PC], bf16, kind="ExternalOutput")

    sup_of = []               # block -> (super idx, kb within super)
    for s, nblk in enumerate(SUPERS):
        for kb in range(nblk):
            sup_of.append((s, kb))
    sup_noff = [int(NOFF[SUP_BLK0[s]]) for s in range(len(SUPERS))]
    sup_w = [int(NOFF[SUP_BLK0[s + 1]] - NOFF[SUP_BLK0[s]])
             for s in range(len(SUPERS))]

    with tile.TileContext(nc) as tc:
        with (
            tc.tile_pool(name="const", bufs=1) as cpool,
            tc.tile_pool(name="inp", bufs=8) as inpp,
            tc.tile_pool(name="aux", bufs=8) as auxp,
            tc.tile_pool(name="xsb", bufs=3) as xsbp,
            tc.tile_pool(name="gact", bufs=6) as gactp,
            tc.tile_pool(name="tmp", bufs=4) as tmpp,
            tc.tile_pool(name="csb", bufs=2) as csbp,
            tc.tile_pool(name="osb", bufs=2) as osbp,
            tc.tile_pool(name="hsb", bufs=2) as hsbp,
            tc.tile_pool(name="ps_x", bufs=2, space="PSUM") as psx,
            tc.tile_pool(name="ps_g", bufs=2, space="PSUM") as psg,
        ):
            cst = cpool.tile([128, CF], bf16)
            cst8 = cpool.tile([128, 4, EMBED], f8)

            stash = {}
            dmas = {}
            sup_state = {}
            pending = []

            def stage_dma(t):
                w = BLOCKS[t]
                o = int(NOFF[t])
                it8 = inpp.tile([128, 4, w], f8, tag="in8")
                nc.sync.dma_start(
                    it8[:], blk8_d[:, 4 * o:4 * (o + w)].rearrange(
                        "p (c n) -> p c n", c=4))
                it16 = inpp.tile([128, 2, w], bf16, tag="in16")
                nc.gpsimd.dma_start(
                    it16[:], blk16_d[:, 2 * o:2 * (o + w)].rearrange(
                        "p (c n) -> p c n", c=2))
                at = auxp.tile([3, w], bf16, tag="aux")
                # aux on Sync right behind its blk: the GpSimd FIFO's out-DMA
                # issues must not delay the x-pass's first (aux) matmul
                nc.sync.dma_start(at[:], aux_d[:, o:o + w])
                dmas[t] = (it8, it16, at)

            def stage_a(t):
                w = BLOCKS[t]
                it, at = dmas.pop(t)
                it8 = it[:, 0:4 * w].bitcast(f8).rearrange(
                    "p (c n) -> p c n", c=4)
                hv = it[:, 4 * w:6 * w].bitcast(bf16)
                cv = it[:, 6 * w:8 * w].bitcast(bf16)
                x_ps = psx.tile([128, w], f32, tag="x")
                for k in range(w // 512):
                    ksl = slice(k * 512, (k + 1) * 512)
                    # aux first: e_v into partitions 64:128, biases everywhere
                    # (a_v sits at 0:64 — DoubleRow dst must start at partition 0)
                    nc.tensor.matmul(x_ps[:, ksl], cst[0:3, CO_WP:CO_WP + 128],
                                     at[:, ksl], start=True, stop=False,
                                     skip_group_check=True)
                    for c0 in (0, 2):  # fp8 DoubleRow: 2 feature chunks per pass
                        nc.tensor.matmul(
                            x_ps[0:64, ksl], cst8[:, c0:c0 + 2, :],
                            it8[:, c0:c0 + 2, ksl], start=False, stop=(c0 == 2),
                            perf_mode=DR, skip_group_check=True)
                x_sb = xsbp.tile([128, w], bf16, tag="xsb")
                nc.vector.tensor_scalar_max(x_sb[:], x_ps[:], 0.0)
                stash[t] = [(hv, cv), x_sb, []]

            def gate(t, j):
                # gate matmuls outrank the (slack-rich) x-pass in the
                # scheduler's priority order, else ACT starves periodically
                with tc.high_priority(offset=60):
                    w = BLOCKS[t]
                    (hv, cv), x_sb, _ = stash[t]
                    s, kb = sup_of[t]
                    gp = psg.tile([128, w], f32, tag="g")
                    for k in range(w // 512):
                        ksl = slice(k * 512, (k + 1) * 512)
                        nc.tensor.matmul(
                            gp[:, ksl],
                            cst[:, CO_WHH + 128 * j:CO_WHH + 128 * (j + 1)],
                            hv[:, ksl], start=True, stop=False)
                        nc.tensor.matmul(
                            gp[:, ksl],
                            cst[:, CO_WIH + 128 * j:CO_WIH + 128 * (j + 1)],
                            x_sb[:, ksl], start=False, stop=True)
                    bias = cst[:, CO_BG + j:CO_BG + j + 1]
                    bo = int(NOFF[t]) - sup_noff[s]
                    if j == 3:  # o-gate straight into the superblock buffer
                        o_sb = sup_state[s][1]
                        nc.scalar.activation(o_sb[:, bo:bo + w], gp[:],
                                             GATE_FUNCS[j], bias=bias)
                    else:
                        ga = gactp.tile([128, w], bf16, tag="ga")
                        stash[t][2].append(ga)
                        nc.scalar.activation(ga[:], gp[:], GATE_FUNCS[j],
                                             bias=bias)

            def finish(s, hsl, last):
                # emitted one block late so c/o are long done when ACT gets here
                c_sb, o_sb, h_sb = sup_state[s]
                w = hsl.stop - hsl.start
                th = tmpp.tile([128, w], bf16, tag="th", bufs=2)
                nc.scalar.activation(th[:], c_sb[:, hsl], AF.Tanh)
                nc.vector.tensor_mul(h_sb[:, hsl], o_sb[:, hsl], th[:])
                # per-slab out-DMAs from the (idle) GpSimd queue: their sem
                # waits must not block the Sync queue's in-DMA prefetch, and
                # per-slab keeps the final drain short
                so = 2 * (sup_noff[s] + hsl.start)
                nc.gpsimd.dma_start(out_d[:, so:so + w], h_sb[:, hsl])
                nc.gpsimd.dma_start(out_d[:, so + w:so + 2 * w], c_sb[:, hsl])

            def stage_b_pre(t):
                s, kb = sup_of[t]
                if kb == 0:
                    w = sup_w[s]
                    c_sb = csbp.tile([128, w], bf16, tag="csb")
                    o_sb = osbp.tile([128, w], bf16, tag="osb")
                    h_sb = hsbp.tile([128, w], bf16, tag="hsb")
                    sup_state[s] = (c_sb, o_sb, h_sb)
                gate(t, 0)
                gate(t, 1)
                gate(t, 2)
                gate(t, 3)

            def stage_b_post(t):
                for fn in pending:
                    fn()
                pending.clear()
                w = BLOCKS[t]
                (hv, cv), x_sb, gact = stash.pop(t)
                s, kb = sup_of[t]
                c_sb, o_sb, h_sb = sup_state[s]
                bo = int(NOFF[t]) - sup_noff[s]
                i_s, f_s, g_t = gact

                t1 = tmpp.tile([128, w], bf16, tag="t1")
                t2 = tmpp.tile([128, w], bf16, tag="t2")
                nc.vector.tensor_mul(t1[:], f_s[:], cv)
                nc.vector.tensor_mul(t2[:], i_s[:], g_t[:])
                nc.vector.tensor_add(c_sb[:, bo:bo + w], t1[:], t2[:])

                # tanh + h in 2-block slabs, deferred one block
                last = kb == SUPERS[s] - 1
                if kb % 2 == 1 or (last and SUPERS[s] % 2 == 1):
                    sb0 = int(NOFF[int(SUP_BLK0[s]) + (kb // 2) * 2]) - sup_noff[s]
                    hsl = slice(sb0, bo + w)
                    pending.append(lambda s=s, hsl=hsl, last=last:
                                   finish(s, hsl, last))

            stage_dma(0)
            nc.sync.dma_start(cst[:], cst_d[:])
            nc.sync.dma_start(cst8[:],
                              cst8_d[:].rearrange("p (c m) -> p c m", c=4))
            stage_dma(1)
            stage_dma(2)
            # DMA-independent dummy matmuls during the fill: keep the PE's
            # HAM activity window busy so the clock gate is at 8/8 (2.4GHz)
            # by the time real data lands (cold MMs run at half rate)
            wsrc = cpool.tile([128, 512], bf16)
            nc.vector.memset(wsrc[:], 0.0)
            warm = psx.tile([64, 512], f32, tag="x")
            for _ in range(10):
                nc.tensor.matmul(warm[:], wsrc[0:2, 0:64],
                                 wsrc[0:2, :], start=True, stop=True)
            for t in range(NBLK + 1):
                if t >= 1:
                    stage_b_pre(t - 1)
                if t + 3 < NBLK:
                    stage_dma(t + 3)
                if t == 1:
                    # block 0 is small: keep its 4 gate ACTs back-to-back
                    # instead of parking g2/g3 behind block 1's x-pass
                    stage_b_post(0)
                    stage_a(1)
                    continue
                if t < NBLK:
                    stage_a(t)
                if t >= 1:
                    stage_b_post(t - 1)
            for fn in pending:
                fn()
            pending.clear()

    nc.finalize()
    return nc


def _pack_cst(W_pos, b_pos, W_hid, b_hid, W_ih, b_ih, W_hh, b_hh):
    # device x layout: partitions 0:64 = a_v, 64:128 = e_v (DoubleRow wants
    # a_v at partition 0) -> swap W_ih.T row halves to match
    cst = np.zeros((128, CF), dtype=np.float32)
    wih_t = W_ih.T                                  # [128, 512]
    cst[0:64, CO_WIH:CO_WIH + 512] = wih_t[64:128]  # a_v rows
    cst[64:128, CO_WIH:CO_WIH + 512] = wih_t[0:64]  # e_v rows
    cst[:, CO_WHH:CO_WHH + 512] = W_hh.T
    bg = b_ih + b_hh
    cst[:, CO_BG:CO_BG + 4] = bg.reshape(4, 128).T
    cst[0:2, CO_WP + 64:CO_WP + 128] = W_pos.T      # [2, 64] -> e_v zone
    cst[2, CO_WP + 64:CO_WP + 128] = b_pos
    cst[2, CO_WP:CO_WP + 64] = b_hid                # a_v zone bias
    # fp8 W_hid.T chunks: cst8[k, c, m] = W_hid[m, 128c + k]
    cst8 = np.ascontiguousarray(
        W_hid.T.reshape(4, 128, EMBED).transpose(1, 0, 2)).astype(nf8)
    return cst.astype(nbf16), cst8.reshape(128, 4 * EMBED)


def _stage_chunk(idxc, Hv_t, hvv_t, xv_t, hv_tm1, cv_tm1, cst, cst8):
    """Gather rows idxc (padded to CAP), pack per-core DRAM images."""
    npad = CAP - len(idxc)
    ic = np.concatenate([idxc, np.zeros(npad, dtype=idxc.dtype)]) if npad else idxc

    hvv_g = hvv_t[ic].astype(nf8)                   # [CAP, 256]
    Hv_g = Hv_t[ic].astype(nf8)
    hv_g = hv_tm1[ic].astype(nbf16)                 # [CAP, 128]
    cv_g = cv_tm1[ic].astype(nbf16)
    aux_g = np.empty((3, CAP), dtype=nbf16)
    aux_g[0:2] = xv_t[ic].T
    aux_g[2] = np.ones(CAP, dtype=nbf16)

    in_maps = []
    for s in range(NCORES):
        sl = slice(s * CAP_PC, (s + 1) * CAP_PC)
        XT = np.empty((512, CAP_PC), dtype=nf8)     # feature-major
        XT[0:256] = hvv_g[sl].T
        XT[256:] = Hv_g[sl].T
        X4 = XT.reshape(4, 128, CAP_PC)
        hvT = hv_g[sl].T                            # [128, CAP_PC]
        cvT = cv_g[sl].T
        blk = np.empty((128, 8 * CAP_PC), dtype=np.uint8)
        for t in range(NBLK):
            o, w = int(NOFF[t]), BLOCKS[t]
            b0 = 8 * o
            blk[:, b0:b0 + 4 * w] = X4[:, :, o:o + w].transpose(
                1, 0, 2).reshape(128, 4 * w).view(np.uint8)
            blk[:, b0 + 4 * w:b0 + 6 * w] = \
                np.ascontiguousarray(hvT[:, o:o + w]).view(np.uint8)
            blk[:, b0 + 6 * w:b0 + 8 * w] = \
                np.ascontiguousarray(cvT[:, o:o + w]).view(np.uint8)
        in_maps.append(dict(blk=blk,
                            aux=np.ascontiguousarray(aux_g[:, sl]),
                            cst=cst, cst8=cst8))
    return in_maps


def _slabs():
    """(abs node offset, width) of each tanh/output slab, mirroring stage_b."""
    out = []
    for su in range(len(SUPERS)):
        b0, nblk = int(SUP_BLK0[su]), SUPERS[su]
        for kb in range(nblk):
            if kb % 2 == 1 or (kb == nblk - 1 and nblk % 2 == 1):
                a = int(NOFF[b0 + (kb // 2) * 2])
                out.append((a, int(NOFF[b0 + kb + 1]) - a))
    return out


def _unpack_chunk(results):
    """Per-core device outputs -> [rows, 128] f32 h and c in gathered order."""
    h_all = np.empty((NCORES * CAP_PC, NODE_H), dtype=np.float32)
    c_all = np.empty((NCORES * CAP_PC, NODE_H), dtype=np.float32)
    for s in range(NCORES):
        o = np.asarray(results[s]["hc_out"])        # [128, 2*CAP_PC] bf16
        r0 = s * CAP_PC
        for a, w in _slabs():
            so = 2 * a
            h_all[r0 + a:r0 + a + w] = o[:, so:so + w].T.astype(np.float32)
            c_all[r0 + a:r0 + a + w] = o[:, so + w:so + 2 * w].T.astype(np.float32)
    return h_all, c_all


def run(inputs, trace=False, tmpdir=None):
    """Stage, run on 8 cores, unstage. Returns ((hv_t, cv_t), BassKernelResults)."""
    inputs = {k: np.asarray(v) for k, v in inputs.items()}
    cst, cst8 = _pack_cst(inputs["W_pos"], inputs["b_pos"], inputs["W_hid"],
                          inputs["b_hid"], inputs["W_ih"], inputs["b_ih"],
                          inputs["W_hh"], inputs["b_hh"])
    idx = np.flatnonzero(inputs["ts_mask"][:, 0] == 1)

    hv_out = inputs["hv_tm1"].astype(np.float32, copy=True)
    cv_out = inputs["cv_tm1"].astype(np.float32, copy=True)

    if "nc" not in _cached:
        _cached["nc"] = build_nc()

    res = None
    for c0 in range(0, max(len(idx), 1), CAP):
        idxc = idx[c0:c0 + CAP]
        in_maps = _stage_chunk(idxc, inputs["Hv_t"], inputs["hvv_t"],
                               inputs["xv_t"], inputs["hv_tm1"],
                               inputs["cv_tm1"], cst, cst8)
        res = run_bass_kernel_spmd(_cached["nc"], in_maps,
                                   core_ids=list(range(NCORES)),
                                   trace=trace, tmpdir=tmpdir)
        if len(idxc):
            h_all, c_all = _unpack_chunk(res.results)
            hv_out[idxc] = h_all[:len(idxc)]
            cv_out[idxc] = c_all[:len(idxc)]
    return (hv_out, cv_out), res


def kernel(**inputs):
    out, _ = run(inputs, trace=False)
    return out
